# revision 48
# baseline (speedup 1.0000x reference)
"""Multi-head self-attention (RoPE + softmax + out-proj) for Trainium2,
sharded over 8 NeuronCores: data-parallel over batch (4) x tensor-parallel
over heads (2 groups of 8). Each core computes q/k/v projections for its
head group, attention, and a partial output projection; the host sums the
two partials per batch and adds the bias.

Layout highlights (433us vs the 557us baseline):
  - x is DMA'd ONCE (f32r bitcast views, fast non-cast HWDGE queues into a
    2-slot landing ring) and cast on the prologue-idle ACT engine to a
    bf16 SBUF-resident copy used by all 4 head-pair projection passes
    (-24MB DMA, no x-reload stalls). All weights arrive bf16 from the
    host: bf16 moving operands stream the PE at 213ns/512 cols where f32r
    takes 427ns (4B/elem SBUF-read bound), halving projection PE time.
  - v is cast straight from its projection PSUM tile into ve_all
    [128, h, mb, 65] (ones column memset once) -- no DRAM bounce buffer.
    The prologue's vproj accumulators alternate onto the then-idle attnV
    PSUM banks (4-deep effective ring) so the PE never waits on the
    ACT-side cast drain; pair-0 projection PSUM rides the scores slots.
  - the softmax pipeline runs at single-mb granularity: a fresh 2-bank
    scores tile per mb from a 2-slot ring (ring WAR lands on exp(mb-2)),
    with a 1024-wide exp per mb, so ACT never sits in the serial
    exp->scores->exp loop that paced the baseline. Steady state is
    ACT-bound at ~1.10us/mb with the PE (scores pair row-group-overlapped
    + 2 attnV matmuls + fillers) fitting underneath.
  - scores are computed transposed (S^T[m, n]) with K=64 row-group-packed
    matmul pairs (two heads concurrently in the PE array); softmax's sum
    over keys m is a matmul reduction via the ones column (M=65); attnV
    runs one mb behind its exp. 1/den is ln+exp(-x) on ACT writing bf16
    straight into the K=1 broadcast matmul (DVE InstReciprocal is 3.3us).
  - RoPE stays f32 out of PSUM on the DVE (bf16 DVE ops measure *slower*
    than f32 on this hardware; the cost model's 2x/4x 16-bit modes do not
    materialize).
  - filler scheduling: the next pair's two projection blocks per quarter
    pop at mb 1 of the following quarter, hidden behind the boundary
    ln/recip chain plus two queued exps; the deferred broadcast+normalize
    pops at mb 6 (mb 2 in the last pair, ahead of the outproj drain);
    out-projection blocks drain at odd mb with the final quarter's spills
    split across DVE and the then-idle ACT. A memset-fed dummy-matmul
    warmup ramps the PE p-state during the initial DMA wait.
  - PSUM: 2x2-bank scores ring, 2 attnV accumulator banks, 2-slot aux
    ring for projection / out-projection / broadcast tiles.
"""

import numpy as np

import concourse.bass as bass
import concourse.mybir as mybir
import concourse.tile as tile

B, N, DIM, H, DH = 4, 2048, 1024, 16, 64
SCALE = DH**-0.5
N_CORES = 8
HG = 8  # heads per core
INNER = HG * DH  # 512, inner dim slice per core
PAIRS = INNER // 128  # 4 head pairs (=128-partition inner chunks)
NB = 4  # n blocks of 512
MB = 16  # m blocks of 128
KD = DIM // 128  # 8 contraction chunks

F32 = mybir.dt.float32
F32R = mybir.dt.float32r
BF16 = mybir.dt.bfloat16
EXP = mybir.ActivationFunctionType.Exp
LOG = mybir.ActivationFunctionType.Ln

MAX_WAITS = 1
WARMUP_MM = 16


def _split_excess_waits(nc):
    """This walrus build rejects >1 semaphore wait per instruction; hoist
    excess waits onto nops inserted before the instruction on its engine."""
    import bass_rust

    for f in nc.m.functions:
        for bb in f.blocks:
            il = bb.instructions
            i = 0
            while i < len(il):
                inst = il[i]
                si = inst.sync_info
                if si is not None and si.on_wait and len(si.on_wait) > MAX_WAITS:
                    waits = list(si.on_wait)
                    si.on_wait = waits[:MAX_WAITS]
                    rest = waits[MAX_WAITS:]
                    eng = nc.engines[inst.engine]
                    insert_at = i
                    for j in range(0, len(rest), MAX_WAITS):
                        b = eng.nop(nofuse=True, hint="wait_split")
                        ni = b.ins
                        tail = nc.cur_bb.bb.instructions
                        assert tail[-1] is ni
                        tail.pop()
                        nsi = ni.sync_info
                        if nsi is None:
                            ni.sync_info = bass_rust.SyncInfo(
                                on_wait=rest[j : j + MAX_WAITS], on_update=[]
                            )
                        else:
                            nsi.on_wait = rest[j : j + MAX_WAITS]
                        il.insert(insert_at, ni)
                        insert_at += 1
                        i += 1
                i += 1


class _FixedTileContext(tile.TileContext):
    def __exit__(self, exc_type, exc_val, exc_tb):
        res = super().__exit__(exc_type, exc_val, exc_tb)
        if exc_type is None:
            _split_excess_waits(self.nc)
        return res


def build_kernel():
    nc = bass.Bass()
    xT = nc.dram_tensor("xT", [DIM, N], F32, kind="ExternalInput")
    # weights arrive pre-cast to bf16 from the host: halves their DMA and,
    # critically, bf16 moving operands stream the PE at 213ns/512-col vs
    # 427ns for f32r (f32r matmuls are SBUF-bandwidth-bound at 4B/elem)
    wq = nc.dram_tensor("wq", [DIM, INNER], BF16, kind="ExternalInput")
    wk = nc.dram_tensor("wk", [DIM, INNER], BF16, kind="ExternalInput")
    wv = nc.dram_tensor("wv", [DIM, INNER], BF16, kind="ExternalInput")
    wo = nc.dram_tensor("wo", [INNER, DIM], BF16, kind="ExternalInput")
    cosT = nc.dram_tensor("cosT", [128, N], F32, kind="ExternalInput")
    sinT = nc.dram_tensor("sinT", [128, N], F32, kind="ExternalInput")
    out = nc.dram_tensor("out", [N, DIM], F32, kind="ExternalOutput")

    # f32 -> f32r is a bit-identical reinterpret, so the x loads can use the
    # fast non-cast HWDGE queues via a bitcast view; x is then cast once to
    # bf16 on the (prologue-idle) ACT engine for all projection matmuls
    xTr = xT.bitcast(F32R).rearrange("(c p) n -> p c n", p=128)

    with _FixedTileContext(nc) as tc:
        with (
            tc.tile_pool(name="const", bufs=1) as cpool,
            tc.tile_pool(name="qk", bufs=1) as qkpool,
            tc.tile_pool(name="ps", space=bass.MemorySpace.PSUM, bufs=1) as ps,
            tc.tile_pool(name="io", bufs=1) as iopool,
            tc.tile_pool(name="xres", bufs=1) as xpool,
        ):
            # ---- constants ----
            # cos/sin tiles are filled in nb-sized slices interleaved behind
            # each nb's x chunks (below) so they never delay the first x/wv
            # chunks that gate the v-projection start
            cos_t = cpool.tile([128, N], F32, tag="cos")
            sin_t = cpool.tile([128, N], F32, tag="sin")
            ones1 = cpool.tile([128, 64], BF16, tag="onesb")
            nc.vector.memset(ones1[:], 1.0)

            # v in attnV layout: [keys-in-mb, head, mb, dh+ones]; written
            # directly from the v-projection PSUM tiles, ones column once
            ve_all = cpool.tile([128, HG, MB, 65], BF16, tag="veall")
            nc.gpsimd.memset(ve_all[:, :, :, 64:65], 1.0)

            # x stays resident (bf16) for all projection passes; the f32r DMA
            # landing tiles are a transient 2-slot ring feeding the casts
            x_res = [
                xpool.tile([128, KD, 512], BF16, tag=f"x{nb}", name=f"xb_{nb}")
                for nb in range(NB)
            ]

            # ---- PE p-state warmup: dummy matmuls with no DMA deps ----
            wu_w = cpool.tile([128, 128], BF16, tag="wuw")
            nc.vector.memset(wu_w[:], 0.0)
            wu_m = cpool.tile([128, 512], BF16, tag="wum")
            nc.vector.memset(wu_m[:], 0.0)
            wu_ps = ps.tile([128, 2, 512], F32, tag="s", bufs=2, name="warm")
            for i in range(WARMUP_MM):
                nc.tensor.matmul(
                    wu_ps[:, i % 2, :], wu_w[:], wu_m[:], start=True, stop=True
                )

            # ---- per-pair q/k projection pieces (half-block granularity so
            #      the attention loop can pop one piece per wave) ----
            def proj_pair_blocks(p, pq_tag="aux"):
                csl = slice(p * 128, (p + 1) * 128)
                wt = {}

                def load_w():
                    for nm, wd in (("q", wq), ("k", wk)):
                        t = iopool.tile(
                            [128, KD, 128], BF16, tag=f"w{nm}", bufs=2,
                            name=f"w{nm}_{p}",
                        )
                        nc.gpsimd.dma_start(
                            t[:],
                            wd.rearrange("(c p) i -> p c i", p=128)[:, :, csl],
                        )
                        wt[nm] = t

                qT_t = qkpool.tile([128, N], BF16, tag="qT", bufs=2)
                kT_t = qkpool.tile([128, N], BF16, tag="kT", bufs=2)

                def block(nb, nm, tgt):
                    def emit():
                        x_t = x_res[nb]
                        nsl = slice(nb * 512, (nb + 1) * 512)
                        pq = ps.tile(
                            [128, 512], F32, tag=pq_tag, bufs=2,
                            name=f"pq_{p}_{nm}{nb}",
                        )
                        for dc in range(KD):
                            nc.tensor.matmul(
                                pq[:], wt[nm][:, dc, :], x_t[:, dc, :],
                                start=(dc == 0), stop=(dc == KD - 1),
                            )
                        # rotate_half via 32-partition shifted copies; sign
                        # folded into sin_t (host negates low half rows).
                        # all-f32 on the DVE: bf16 DVE ops are slower on this
                        # hardware, and the PSUM source excludes gpsimd.
                        tmp = iopool.tile([128, 512], F32, tag="tmp", bufs=2)
                        for g in range(4):
                            dst = slice(g * 32, (g + 1) * 32)
                            ssrc = slice((g ^ 1) * 32, ((g ^ 1) + 1) * 32)
                            nc.vector.tensor_copy(tmp[dst, :], pq[ssrc, :])
                        nc.vector.tensor_mul(tmp[:], tmp[:], sin_t[:, nsl])
                        nc.vector.tensor_mul(tgt[:, nsl], pq[:], cos_t[:, nsl])
                        nc.vector.tensor_add(tgt[:, nsl], tgt[:, nsl], tmp[:])
                    return emit

                pieces = []
                for nb in range(NB):
                    for nm, tgt in (("q", qT_t), ("k", kT_t)):
                        pieces.append(block(nb, nm, tgt))
                return load_w, pieces, qT_t, kT_t

            # pair-0's projection PSUM rides the (prologue-idle) scores slots
            # so it never contends with the vproj ring
            load_w0, pieces0, qT0, kT0 = proj_pair_blocks(0, pq_tag="s")
            load_w0()

            def _pair0_emit(nb):
                pieces0[2 * nb]()
                pieces0[2 * nb + 1]()

            # ---- first pass over x: v projection (all heads) + pair-0 q/k ----
            pair0_hook = {"emit": _pair0_emit}
            with tc.tile_pool(name="vproj", bufs=1) as vpj:
                wv_t = vpj.tile([128, KD, INNER], BF16, tag="wv")
                wvr = wv.rearrange("(c p) i -> p c i", p=128)
                for nb in range(NB):
                    xf_t = vpj.tile(
                        [128, KD, 512], F32R, tag="xf", bufs=2,
                        name=f"xf_{nb}",
                    )
                    for dc in range(KD):
                        # interleave the wv chunks with the first x tile so the
                        # accumulation chain can start as soon as chunk 0 lands
                        if nb == 0:
                            weng = nc.scalar if dc % 2 == 0 else nc.sync
                            weng.dma_start(wv_t[:, dc, :], wvr[:, dc, :])
                        eng = nc.sync if dc % 2 == 0 else nc.scalar
                        eng.dma_start(
                            xf_t[:, dc, :],
                            xTr[:, dc, nb * 512 : (nb + 1) * 512],
                        )
                    nbs = slice(nb * 512, (nb + 1) * 512)
                    nc.sync.dma_start(cos_t[:, nbs], cosT[:, nbs])
                    nc.scalar.dma_start(sin_t[:, nbs], sinT[:, nbs])
                    xv_t = x_res[nb]
                    for dc2 in range(KD // 2):
                        # chunk-pair casts pipeline behind the DMAs
                        nc.scalar.copy(
                            xv_t[:, 2 * dc2 : 2 * dc2 + 2, :],
                            xf_t[:, 2 * dc2 : 2 * dc2 + 2, :],
                        )
                    for sub in range(4):
                        # alternate the (prologue-idle) attnV accumulator
                        # slots with aux: a 4-deep effective ring so the PE
                        # never waits on the ACT-side v-cast drain
                        pv = ps.tile(
                            [128, 512], F32,
                            tag="aux" if sub % 2 == 0 else "ot", bufs=2,
                        )
                        for dc in range(KD):
                            nc.tensor.matmul(
                                pv[:],
                                xv_t[:, dc, sub * 128 : (sub + 1) * 128],
                                wv_t[:, dc, :],
                                start=(dc == 0),
                                stop=(dc == KD - 1),
                            )
                        # cast straight into the attnV stationary layout (ACT
                        # engine: idle during the prologue, and gpsimd cannot
                        # read PSUM)
                        mb = nb * 4 + sub
                        nc.scalar.copy(
                            ve_all[:, 0:HG, mb, 0:64],
                            pv[:].rearrange("p (h d) -> p h d", h=HG),
                        )
                    pair0_hook["emit"](nb)

            pair_qk = {0: (qT0, kT0)}

            # ---- attention (pair p) interleaved with projections (p+1) ----
            with tc.tile_pool(name="attn", bufs=1) as at:
                otn = [
                    at.tile([128, 4, 512], BF16, tag=f"otn{p}", name=f"otn{p}")
                    for p in range(PAIRS)
                ]
                wo_h = []

                def load_wo():
                    # wo arrives bf16 from the host: straight DMA, no staging
                    for dh, wtag in ((0, "qT"), (1, "kT")):
                        woh = qkpool.tile(
                            [128, PAIRS, 512], BF16, tag=wtag, bufs=2,
                            name=f"wo_h{dh}",
                        )
                        nc.gpsimd.dma_start(
                            woh[:],
                            wo.rearrange("(c p) d -> p c d", p=128)[
                                :, :, dh * 512 : (dh + 1) * 512
                            ],
                        )
                        wo_h.append(woh)

                opq = []

                def outproj_block(nb, dh, on_act=False):
                    def emit():
                        q4, r4 = divmod(nb, 4)
                        nsl = slice(nb * 128, (nb + 1) * 128)
                        po = ps.tile([128, 512], F32, tag="aux", bufs=2)
                        for c in range(PAIRS):
                            nc.tensor.matmul(
                                po[:],
                                otn[c][:, q4, r4 * 128 : (r4 + 1) * 128],
                                wo_h[dh][:, c, :],
                                start=(c == 0),
                                stop=(c == PAIRS - 1),
                            )
                        ost = iopool.tile([128, 512], F32, tag="ost", bufs=3)
                        if on_act:
                            # final drain only: ACT is idle then, splitting
                            # the PSUM spills across two engines
                            nc.scalar.copy(ost[:], po[:])
                        else:
                            nc.vector.tensor_copy(ost[:], po[:])
                        nc.sync.dma_start(
                            out[nsl, dh * 512 : (dh + 1) * 512], ost[:]
                        )
                    return emit

                def outproj_quarter(q4, final=False):
                    # queue this quarter's out-projection; drained one block
                    # at a time inside the next quarter's attention loop (the
                    # final quarter alternates its PSUM spills onto the
                    # then-idle ACT engine)
                    for i, (r4, dh) in enumerate(
                        (r4, dh) for r4 in range(4) for dh in range(2)
                    ):
                        opq.append(
                            outproj_block(q4 * 4 + r4, dh, final and i % 2 == 1)
                        )

                pending_norm = {"fn": None}
                for p in range(PAIRS):
                    qT_t, kT_t = pair_qk.pop(p)
                    if p == PAIRS - 1:
                        load_wo()
                    if p + 1 < PAIRS:
                        load_wn, pieces_n, qTn, kTn = proj_pair_blocks(p + 1)
                        load_wn()
                        pair_qk[p + 1] = (qTn, kTn)
                    else:
                        pieces_n = []
                    blk_i = 0
                    for f in range(2):
                        for sub in range(2):
                            n0 = f * 1024 + sub * 512
                            ot_ab = [
                                ps.tile([128, 512], F32, tag="ot", bufs=2, name=f"ot{jj}")
                                for jj in range(2)
                            ]

                            def attn_mm(pt2_l, mb_l):
                                for j in range(2):
                                    nc.tensor.matmul(
                                        ot_ab[j][0:65, :],
                                        ve_all[:, 2 * p + j, mb_l, :],
                                        pt2_l[:, j, :],
                                        start=(mb_l == 0),
                                        stop=(mb_l == MB - 1),
                                    )

                            pend = None
                            for mb in range(MB):
                                # fresh 2-bank scores tile per mb from a
                                # 2-slot ring: the ring WAR points at the exp
                                # of mb-2 (long done), so the PE never waits
                                # on the previous exp (PSUM dependency
                                # tracking is whole-tile, so an in-place
                                # half-ping-pong does NOT break that chain)
                                s2 = ps.tile(
                                    [128, 2, 512], F32, tag="s", bufs=2,
                                    name="s2",
                                )
                                msl = slice(mb * 128, (mb + 1) * 128)
                                # j-inner: the two K=64 matmuls alternate PE
                                # row groups and stream concurrently
                                for j in range(2):
                                    psl = slice(64 * j, 64 * (j + 1))
                                    nc.tensor.matmul(
                                        s2[:, j, :],
                                        kT_t[psl, msl],
                                        qT_t[psl, n0 : n0 + 512],
                                        start=True,
                                        stop=True,
                                    )
                                pt2 = at.tile(
                                    [128, 2, 512], BF16, tag="pt", bufs=8,
                                    name="pt2",
                                )
                                nc.scalar.activation(
                                    pt2[:], s2[:], EXP, scale=SCALE
                                )
                                # attnV runs one mb behind its exp so the PE
                                # never queues a matmul whose pt isn't ready
                                if pend is not None:
                                    attn_mm(*pend)
                                pend = (pt2, mb)
                                # the previous quarter's two projection
                                # filler blocks land at mb 1: the PE hides
                                # them behind the ~5us ACT backlog of the
                                # boundary ln/recip chain plus the first two
                                # queued exps
                                # (the pair's first quarter pops at mb 9
                                # instead, giving the gpsimd weight DMA
                                # issued at pair start time to land)
                                first_q = f == 0 and sub == 0
                                if mb == (9 if first_q else 1):
                                    for _ in range(2):
                                        if blk_i < len(pieces_n):
                                            pieces_n[blk_i]()
                                            blk_i += 1
                                # previous quarter's deferred broadcast +
                                # normalize: by now its denominator recip is
                                # long done. In pairs with filler blocks the
                                # pop waits until mb 6 so the bct never queues
                                # behind a still-open pq on the aux ring; the
                                # last pair (whose outproj pops read otn from
                                # mb 3) keeps it at mb 2.
                                norm_mb = 2 if p == PAIRS - 1 else 6
                                if mb == norm_mb and pending_norm["fn"]:
                                    pending_norm["fn"]()
                                    pending_norm["fn"] = None
                                # keep the PE dense without bubbling the exp
                                # pipeline: the two projection filler blocks
                                # per quarter land at the quarter boundary
                                # (mb 15), where the ln/recip chain gives the
                                # PE a free window; out-projection blocks in
                                # the last pair drain at every odd mb (only
                                # after mb 2's deferred norm, which writes the
                                # otn regions outproj reads)
                                if mb % 2 == 1 and mb > 2 and opq:
                                    opq.pop(0)()
                                if mb == 14 and len(opq) > 6:
                                    opq.pop(0)()
                            attn_mm(*pend)
                            # normalization: spill O rows; 1/den via ln per
                            # head + one combined exp(-x) on ACT writing bf16
                            # (feeds the K=1 broadcast matmul); frees both
                            # "ot" banks right away, broadcast + one DVE mul
                            # deferred into the next quarter.
                            osb = at.tile([64, 2, 512], F32, tag="ots", bufs=4)
                            recbs = []
                            for j in range(2):
                                lnd = at.tile(
                                    [1, 512], F32, tag="lnd", bufs=4,
                                    name=f"lnd_{j}",
                                )
                                nc.scalar.activation(
                                    lnd[:], ot_ab[j][64:65, :], LOG
                                )
                                nc.vector.tensor_copy(
                                    osb[:, j, :], ot_ab[j][0:64, :]
                                )
                                recb = at.tile(
                                    [1, 512], BF16, tag="recb", bufs=4,
                                    name=f"recb_{j}",
                                )
                                nc.scalar.activation(
                                    recb[:], lnd[:], EXP, scale=-1.0
                                )
                                recbs.append(recb)

                            def make_norm(osb_l, recb_l, p_l, qi_l):
                                def go():
                                    for j in range(2):
                                        bct = ps.tile(
                                            [128, 512], F32, tag="aux",
                                            bufs=2, name=f"bct{j}",
                                        )
                                        nc.tensor.matmul(
                                            bct[0:64, :],
                                            ones1[0:1, :],
                                            recb_l[j][:],
                                            start=True,
                                            stop=True,
                                        )
                                        nc.vector.tensor_mul(
                                            otn[p_l][
                                                64 * j : 64 * (j + 1), qi_l, :
                                            ],
                                            osb_l[:, j, :],
                                            bct[0:64, :],
                                        )
                                return go

                            pending_norm["fn"] = make_norm(
                                osb, recbs, p, f * 2 + sub
                            )
                            if p == PAIRS - 1:
                                last = f == 1 and sub == 1
                                if last:
                                    pending_norm["fn"]()
                                    pending_norm["fn"] = None
                                outproj_quarter(f * 2 + sub, final=last)
                                if last:
                                    while opq:
                                        opq.pop(0)()

    return nc


_CACHED = {}


def _get_kernel():
    if "nc" not in _CACHED:
        _CACHED["nc"] = build_kernel()
    return _CACHED["nc"]


def kernel(x, rotary_emb_x, Wq, Wkv, Wo, bo):
    import ml_dtypes
    from concourse.bass_utils import run_bass_kernel_spmd

    x = np.asarray(x, np.float32)
    rope = np.asarray(rotary_emb_x, np.float32)
    Wq = np.asarray(Wq, np.float32).astype(ml_dtypes.bfloat16)
    Wkv = np.asarray(Wkv, np.float32).astype(ml_dtypes.bfloat16)
    Wo = np.asarray(Wo, np.float32).astype(ml_dtypes.bfloat16)
    bo = np.asarray(bo, np.float32)

    cosT = np.ascontiguousarray(np.cos(rope).T)  # [64, N]
    sinT = np.ascontiguousarray(np.sin(rope).T)
    cosT2 = np.ascontiguousarray(np.concatenate([cosT, cosT], axis=0))
    sinT2 = np.concatenate([sinT, sinT], axis=0)
    # fold rotate_half's sign into sin: the low half of each 64-row head
    # block multiplies -q_hi
    sinT2 = sinT2.copy()
    sinT2[0:32] = -sinT2[0:32]
    sinT2[64:96] = -sinT2[64:96]
    sinT2 = np.ascontiguousarray(sinT2)

    Wk_full = Wkv[:, : H * DH]
    Wv_full = Wkv[:, H * DH :]

    xTs = [np.ascontiguousarray(x[b].T) for b in range(B)]
    in_maps = []
    for core in range(N_CORES):
        b, hg = divmod(core, 2)
        isl = slice(hg * INNER, (hg + 1) * INNER)
        in_maps.append(
            {
                "xT": xTs[b],
                "wq": np.ascontiguousarray(Wq[:, isl]),
                "wk": np.ascontiguousarray(Wk_full[:, isl]),
                "wv": np.ascontiguousarray(Wv_full[:, isl]),
                "wo": np.ascontiguousarray(Wo[isl, :]),
                "cosT": cosT2,
                "sinT": sinT2,
            }
        )

    nc = _get_kernel()
    _CACHED["in_maps"] = in_maps
    res = run_bass_kernel_spmd(nc, in_maps, list(range(N_CORES)))
    outs = [res.results[i]["out"] for i in range(N_CORES)]
    full = np.stack(
        [outs[2 * b] + outs[2 * b + 1] + bo for b in range(B)], axis=0
    )
    return full


# revision 50
# speedup vs baseline: 1.0009x; 1.0009x over previous
"""Multi-head self-attention (RoPE + softmax + out-proj) for Trainium2,
sharded over 8 NeuronCores: data-parallel over batch (4) x tensor-parallel
over heads (2 groups of 8). Each core computes q/k/v projections for its
head group, attention, and a partial output projection; the host sums the
two partials per batch and adds the bias.

Layout highlights (433us vs the 557us baseline):
  - x is DMA'd ONCE (f32r bitcast views, fast non-cast HWDGE queues into a
    2-slot landing ring) and cast on the prologue-idle ACT engine to a
    bf16 SBUF-resident copy used by all 4 head-pair projection passes
    (-24MB DMA, no x-reload stalls). All weights arrive bf16 from the
    host: bf16 moving operands stream the PE at 213ns/512 cols where f32r
    takes 427ns (4B/elem SBUF-read bound), halving projection PE time.
  - v is cast straight from its projection PSUM tile into ve_all
    [128, h, mb, 65] (ones column memset once) -- no DRAM bounce buffer.
    The prologue's vproj accumulators alternate onto the then-idle attnV
    PSUM banks (4-deep effective ring) so the PE never waits on the
    ACT-side cast drain; pair-0 projection PSUM rides the scores slots.
  - the softmax pipeline runs at single-mb granularity: a fresh 2-bank
    scores tile per mb from a 2-slot ring (ring WAR lands on exp(mb-2)),
    with a 1024-wide exp per mb, so ACT never sits in the serial
    exp->scores->exp loop that paced the baseline. Steady state is
    ACT-bound at ~1.10us/mb with the PE (scores pair row-group-overlapped
    + 2 attnV matmuls + fillers) fitting underneath.
  - scores are computed transposed (S^T[m, n]) with K=64 row-group-packed
    matmul pairs (two heads concurrently in the PE array); softmax's sum
    over keys m is a matmul reduction via the ones column (M=65); attnV
    runs one mb behind its exp. 1/den is ln+exp(-x) on ACT writing bf16
    straight into the K=1 broadcast matmul (DVE InstReciprocal is 3.3us).
  - RoPE stays f32 out of PSUM on the DVE (bf16 DVE ops measure *slower*
    than f32 on this hardware; the cost model's 2x/4x 16-bit modes do not
    materialize).
  - filler scheduling: the next pair's two projection blocks per quarter
    pop at mb 1 of the following quarter, hidden behind the boundary
    ln/recip chain plus two queued exps; the deferred broadcast+normalize
    pops at mb 6 (mb 2 in the last pair, ahead of the outproj drain);
    out-projection blocks drain at odd mb with the final quarter's spills
    split across DVE and the then-idle ACT. A memset-fed dummy-matmul
    warmup ramps the PE p-state during the initial DMA wait.
  - PSUM: 2x2-bank scores ring, 2 attnV accumulator banks, 2-slot aux
    ring for projection / out-projection / broadcast tiles.
"""

import numpy as np

import concourse.bass as bass
import concourse.mybir as mybir
import concourse.tile as tile

B, N, DIM, H, DH = 4, 2048, 1024, 16, 64
SCALE = DH**-0.5
N_CORES = 8
HG = 8  # heads per core
INNER = HG * DH  # 512, inner dim slice per core
PAIRS = INNER // 128  # 4 head pairs (=128-partition inner chunks)
NB = 4  # n blocks of 512
MB = 16  # m blocks of 128
KD = DIM // 128  # 8 contraction chunks

F32 = mybir.dt.float32
F32R = mybir.dt.float32r
BF16 = mybir.dt.bfloat16
EXP = mybir.ActivationFunctionType.Exp
LOG = mybir.ActivationFunctionType.Ln

MAX_WAITS = 1
WARMUP_MM = 28


def _split_excess_waits(nc):
    """This walrus build rejects >1 semaphore wait per instruction; hoist
    excess waits onto nops inserted before the instruction on its engine."""
    import bass_rust

    for f in nc.m.functions:
        for bb in f.blocks:
            il = bb.instructions
            i = 0
            while i < len(il):
                inst = il[i]
                si = inst.sync_info
                if si is not None and si.on_wait and len(si.on_wait) > MAX_WAITS:
                    waits = list(si.on_wait)
                    si.on_wait = waits[:MAX_WAITS]
                    rest = waits[MAX_WAITS:]
                    eng = nc.engines[inst.engine]
                    insert_at = i
                    for j in range(0, len(rest), MAX_WAITS):
                        b = eng.nop(nofuse=True, hint="wait_split")
                        ni = b.ins
                        tail = nc.cur_bb.bb.instructions
                        assert tail[-1] is ni
                        tail.pop()
                        nsi = ni.sync_info
                        if nsi is None:
                            ni.sync_info = bass_rust.SyncInfo(
                                on_wait=rest[j : j + MAX_WAITS], on_update=[]
                            )
                        else:
                            nsi.on_wait = rest[j : j + MAX_WAITS]
                        il.insert(insert_at, ni)
                        insert_at += 1
                        i += 1
                i += 1


class _FixedTileContext(tile.TileContext):
    def __exit__(self, exc_type, exc_val, exc_tb):
        res = super().__exit__(exc_type, exc_val, exc_tb)
        if exc_type is None:
            _split_excess_waits(self.nc)
        return res


def build_kernel():
    nc = bass.Bass()
    xT = nc.dram_tensor("xT", [DIM, N], F32, kind="ExternalInput")
    # weights arrive pre-cast to bf16 from the host: halves their DMA and,
    # critically, bf16 moving operands stream the PE at 213ns/512-col vs
    # 427ns for f32r (f32r matmuls are SBUF-bandwidth-bound at 4B/elem)
    wq = nc.dram_tensor("wq", [DIM, INNER], BF16, kind="ExternalInput")
    wk = nc.dram_tensor("wk", [DIM, INNER], BF16, kind="ExternalInput")
    wv = nc.dram_tensor("wv", [DIM, INNER], BF16, kind="ExternalInput")
    wo = nc.dram_tensor("wo", [INNER, DIM], BF16, kind="ExternalInput")
    cosT = nc.dram_tensor("cosT", [128, N], F32, kind="ExternalInput")
    sinT = nc.dram_tensor("sinT", [128, N], F32, kind="ExternalInput")
    out = nc.dram_tensor("out", [N, DIM], F32, kind="ExternalOutput")

    # f32 -> f32r is a bit-identical reinterpret, so the x loads can use the
    # fast non-cast HWDGE queues via a bitcast view; x is then cast once to
    # bf16 on the (prologue-idle) ACT engine for all projection matmuls
    xTr = xT.bitcast(F32R).rearrange("(c p) n -> p c n", p=128)

    with _FixedTileContext(nc) as tc:
        with (
            tc.tile_pool(name="const", bufs=1) as cpool,
            tc.tile_pool(name="qk", bufs=1) as qkpool,
            tc.tile_pool(name="ps", space=bass.MemorySpace.PSUM, bufs=1) as ps,
            tc.tile_pool(name="io", bufs=1) as iopool,
            tc.tile_pool(name="xres", bufs=1) as xpool,
        ):
            # ---- constants ----
            # cos/sin tiles are filled in nb-sized slices interleaved behind
            # each nb's x chunks (below) so they never delay the first x/wv
            # chunks that gate the v-projection start
            cos_t = cpool.tile([128, N], F32, tag="cos")
            sin_t = cpool.tile([128, N], F32, tag="sin")
            ones1 = cpool.tile([128, 64], BF16, tag="onesb")
            nc.vector.memset(ones1[:], 1.0)

            # v in attnV layout: [keys-in-mb, head, mb, dh+ones]; written
            # directly from the v-projection PSUM tiles, ones column once
            ve_all = cpool.tile([128, HG, MB, 65], BF16, tag="veall")
            nc.gpsimd.memset(ve_all[:, :, :, 64:65], 1.0)

            # x stays resident (bf16) for all projection passes; the f32r DMA
            # landing tiles are a transient 2-slot ring feeding the casts
            x_res = [
                xpool.tile([128, KD, 512], BF16, tag=f"x{nb}", name=f"xb_{nb}")
                for nb in range(NB)
            ]

            # ---- PE p-state warmup: dummy matmuls with no DMA deps ----
            wu_w = cpool.tile([128, 128], BF16, tag="wuw")
            nc.vector.memset(wu_w[:], 0.0)
            wu_m = cpool.tile([128, 512], BF16, tag="wum")
            nc.vector.memset(wu_m[:], 0.0)
            wu_ps = ps.tile([128, 2, 512], F32, tag="s", bufs=2, name="warm")
            for i in range(WARMUP_MM):
                nc.tensor.matmul(
                    wu_ps[:, i % 2, :], wu_w[:], wu_m[:], start=True, stop=True
                )

            # ---- per-pair q/k projection pieces (half-block granularity so
            #      the attention loop can pop one piece per wave) ----
            def proj_pair_blocks(p, pq_tag="aux"):
                csl = slice(p * 128, (p + 1) * 128)
                wt = {}

                def load_w():
                    for nm, wd in (("q", wq), ("k", wk)):
                        t = iopool.tile(
                            [128, KD, 128], BF16, tag=f"w{nm}", bufs=2,
                            name=f"w{nm}_{p}",
                        )
                        nc.gpsimd.dma_start(
                            t[:],
                            wd.rearrange("(c p) i -> p c i", p=128)[:, :, csl],
                        )
                        wt[nm] = t

                qT_t = qkpool.tile([128, N], BF16, tag="qT", bufs=2)
                kT_t = qkpool.tile([128, N], BF16, tag="kT", bufs=2)

                def block(nb, nm, tgt):
                    def emit():
                        x_t = x_res[nb]
                        nsl = slice(nb * 512, (nb + 1) * 512)
                        pq = ps.tile(
                            [128, 512], F32, tag=pq_tag, bufs=2,
                            name=f"pq_{p}_{nm}{nb}",
                        )
                        for dc in range(KD):
                            nc.tensor.matmul(
                                pq[:], wt[nm][:, dc, :], x_t[:, dc, :],
                                start=(dc == 0), stop=(dc == KD - 1),
                            )
                        # rotate_half via 32-partition shifted copies; sign
                        # folded into sin_t (host negates low half rows).
                        # all-f32 on the DVE: bf16 DVE ops are slower on this
                        # hardware, and the PSUM source excludes gpsimd.
                        tmp = iopool.tile([128, 512], F32, tag="tmp", bufs=2)
                        for g in range(4):
                            dst = slice(g * 32, (g + 1) * 32)
                            ssrc = slice((g ^ 1) * 32, ((g ^ 1) + 1) * 32)
                            nc.vector.tensor_copy(tmp[dst, :], pq[ssrc, :])
                        nc.vector.tensor_mul(tmp[:], tmp[:], sin_t[:, nsl])
                        nc.vector.tensor_mul(tgt[:, nsl], pq[:], cos_t[:, nsl])
                        nc.vector.tensor_add(tgt[:, nsl], tgt[:, nsl], tmp[:])
                    return emit

                pieces = []
                for nb in range(NB):
                    for nm, tgt in (("q", qT_t), ("k", kT_t)):
                        pieces.append(block(nb, nm, tgt))
                return load_w, pieces, qT_t, kT_t

            # pair-0's projection PSUM rides the (prologue-idle) scores slots
            # so it never contends with the vproj ring
            load_w0, pieces0, qT0, kT0 = proj_pair_blocks(0, pq_tag="s")
            load_w0()

            def _pair0_emit(nb):
                pieces0[2 * nb]()
                pieces0[2 * nb + 1]()

            # ---- first pass over x: v projection (all heads) + pair-0 q/k ----
            pair0_hook = {"emit": _pair0_emit}
            with tc.tile_pool(name="vproj", bufs=1) as vpj:
                wv_t = vpj.tile([128, KD, INNER], BF16, tag="wv")
                wvr = wv.rearrange("(c p) i -> p c i", p=128)
                for nb in range(NB):
                    xf_t = vpj.tile(
                        [128, KD, 512], F32R, tag="xf", bufs=2,
                        name=f"xf_{nb}",
                    )
                    for dc in range(KD):
                        # interleave the wv chunks with the first x tile so the
                        # accumulation chain can start as soon as chunk 0 lands
                        if nb == 0:
                            weng = nc.scalar if dc % 2 == 0 else nc.sync
                            weng.dma_start(wv_t[:, dc, :], wvr[:, dc, :])
                        eng = nc.sync if dc % 2 == 0 else nc.scalar
                        eng.dma_start(
                            xf_t[:, dc, :],
                            xTr[:, dc, nb * 512 : (nb + 1) * 512],
                        )
                    nbs = slice(nb * 512, (nb + 1) * 512)
                    nc.sync.dma_start(cos_t[:, nbs], cosT[:, nbs])
                    nc.scalar.dma_start(sin_t[:, nbs], sinT[:, nbs])
                    xv_t = x_res[nb]
                    for dc2 in range(KD // 2):
                        # chunk-pair casts pipeline behind the DMAs; nb0's go
                        # on the DVE (idle until the first RoPE ~12us in) so
                        # the very first vproj chain isn't gated by ACT
                        ceng = nc.vector if nb == 0 else nc.scalar
                        if nb == 0:
                            ceng.tensor_copy(
                                xv_t[:, 2 * dc2 : 2 * dc2 + 2, :],
                                xf_t[:, 2 * dc2 : 2 * dc2 + 2, :],
                            )
                        else:
                            ceng.copy(
                                xv_t[:, 2 * dc2 : 2 * dc2 + 2, :],
                                xf_t[:, 2 * dc2 : 2 * dc2 + 2, :],
                            )
                    for sub in range(4):
                        # alternate the (prologue-idle) attnV accumulator
                        # slots with aux: a 4-deep effective ring so the PE
                        # never waits on the ACT-side v-cast drain
                        pv = ps.tile(
                            [128, 512], F32,
                            tag="aux" if sub % 2 == 0 else "ot", bufs=2,
                        )
                        for dc in range(KD):
                            nc.tensor.matmul(
                                pv[:],
                                xv_t[:, dc, sub * 128 : (sub + 1) * 128],
                                wv_t[:, dc, :],
                                start=(dc == 0),
                                stop=(dc == KD - 1),
                            )
                        # cast straight into the attnV stationary layout (ACT
                        # engine: idle during the prologue, and gpsimd cannot
                        # read PSUM)
                        mb = nb * 4 + sub
                        nc.scalar.copy(
                            ve_all[:, 0:HG, mb, 0:64],
                            pv[:].rearrange("p (h d) -> p h d", h=HG),
                        )
                    pair0_hook["emit"](nb)

            pair_qk = {0: (qT0, kT0)}

            # ---- attention (pair p) interleaved with projections (p+1) ----
            with tc.tile_pool(name="attn", bufs=1) as at:
                otn = [
                    at.tile([128, 4, 512], BF16, tag=f"otn{p}", name=f"otn{p}")
                    for p in range(PAIRS)
                ]
                wo_h = []

                def load_wo():
                    # wo arrives bf16 from the host: straight DMA, no staging
                    for dh, wtag in ((0, "qT"), (1, "kT")):
                        woh = qkpool.tile(
                            [128, PAIRS, 512], BF16, tag=wtag, bufs=2,
                            name=f"wo_h{dh}",
                        )
                        nc.gpsimd.dma_start(
                            woh[:],
                            wo.rearrange("(c p) d -> p c d", p=128)[
                                :, :, dh * 512 : (dh + 1) * 512
                            ],
                        )
                        wo_h.append(woh)

                opq = []

                def outproj_block(nb, dh, on_act=False):
                    def emit():
                        q4, r4 = divmod(nb, 4)
                        nsl = slice(nb * 128, (nb + 1) * 128)
                        po = ps.tile([128, 512], F32, tag="aux", bufs=2)
                        for c in range(PAIRS):
                            nc.tensor.matmul(
                                po[:],
                                otn[c][:, q4, r4 * 128 : (r4 + 1) * 128],
                                wo_h[dh][:, c, :],
                                start=(c == 0),
                                stop=(c == PAIRS - 1),
                            )
                        ost = iopool.tile([128, 512], F32, tag="ost", bufs=3)
                        if on_act:
                            # final drain only: ACT is idle then, splitting
                            # the PSUM spills across two engines
                            nc.scalar.copy(ost[:], po[:])
                        else:
                            nc.vector.tensor_copy(ost[:], po[:])
                        nc.sync.dma_start(
                            out[nsl, dh * 512 : (dh + 1) * 512], ost[:]
                        )
                    return emit

                def outproj_quarter(q4, final=False):
                    # queue this quarter's out-projection; drained one block
                    # at a time inside the next quarter's attention loop (the
                    # final quarter alternates its PSUM spills onto the
                    # then-idle ACT engine)
                    for i, (r4, dh) in enumerate(
                        (r4, dh) for r4 in range(4) for dh in range(2)
                    ):
                        opq.append(
                            outproj_block(q4 * 4 + r4, dh, final and i % 2 == 1)
                        )

                pending_norm = {"fn": None}
                for p in range(PAIRS):
                    qT_t, kT_t = pair_qk.pop(p)
                    if p == PAIRS - 1:
                        load_wo()
                    if p + 1 < PAIRS:
                        load_wn, pieces_n, qTn, kTn = proj_pair_blocks(p + 1)
                        load_wn()
                        pair_qk[p + 1] = (qTn, kTn)
                    else:
                        pieces_n = []
                    blk_i = 0
                    for f in range(2):
                        for sub in range(2):
                            n0 = f * 1024 + sub * 512
                            ot_ab = [
                                ps.tile([128, 512], F32, tag="ot", bufs=2, name=f"ot{jj}")
                                for jj in range(2)
                            ]

                            def attn_mm(pt2_l, mb_l):
                                for j in range(2):
                                    nc.tensor.matmul(
                                        ot_ab[j][0:65, :],
                                        ve_all[:, 2 * p + j, mb_l, :],
                                        pt2_l[:, j, :],
                                        start=(mb_l == 0),
                                        stop=(mb_l == MB - 1),
                                    )

                            pend = None
                            for mb in range(MB):
                                # fresh 2-bank scores tile per mb from a
                                # 2-slot ring: the ring WAR points at the exp
                                # of mb-2 (long done), so the PE never waits
                                # on the previous exp (PSUM dependency
                                # tracking is whole-tile, so an in-place
                                # half-ping-pong does NOT break that chain)
                                s2 = ps.tile(
                                    [128, 2, 512], F32, tag="s", bufs=2,
                                    name="s2",
                                )
                                msl = slice(mb * 128, (mb + 1) * 128)
                                # j-inner: the two K=64 matmuls alternate PE
                                # row groups and stream concurrently
                                for j in range(2):
                                    psl = slice(64 * j, 64 * (j + 1))
                                    nc.tensor.matmul(
                                        s2[:, j, :],
                                        kT_t[psl, msl],
                                        qT_t[psl, n0 : n0 + 512],
                                        start=True,
                                        stop=True,
                                    )
                                pt2 = at.tile(
                                    [128, 2, 512], BF16, tag="pt", bufs=8,
                                    name="pt2",
                                )
                                nc.scalar.activation(
                                    pt2[:], s2[:], EXP, scale=SCALE
                                )
                                # attnV runs one mb behind its exp so the PE
                                # never queues a matmul whose pt isn't ready
                                if pend is not None:
                                    attn_mm(*pend)
                                pend = (pt2, mb)
                                # the previous quarter's two projection
                                # filler blocks land at mb 1: the PE hides
                                # them behind the ~5us ACT backlog of the
                                # boundary ln/recip chain plus the first two
                                # queued exps
                                # (the pair's first quarter pops at mb 9
                                # instead, giving the gpsimd weight DMA
                                # issued at pair start time to land)
                                first_q = f == 0 and sub == 0
                                if mb == (9 if first_q else 1):
                                    for _ in range(2):
                                        if blk_i < len(pieces_n):
                                            pieces_n[blk_i]()
                                            blk_i += 1
                                # previous quarter's deferred broadcast +
                                # normalize: by now its denominator recip is
                                # long done. In pairs with filler blocks the
                                # pop waits until mb 6 so the bct never queues
                                # behind a still-open pq on the aux ring; the
                                # last pair (whose outproj pops read otn from
                                # mb 3) keeps it at mb 2.
                                norm_mb = 2 if p == PAIRS - 1 else 6
                                if mb == norm_mb and pending_norm["fn"]:
                                    pending_norm["fn"]()
                                    pending_norm["fn"] = None
                                # keep the PE dense without bubbling the exp
                                # pipeline: the two projection filler blocks
                                # per quarter land at the quarter boundary
                                # (mb 15), where the ln/recip chain gives the
                                # PE a free window; out-projection blocks in
                                # the last pair drain at every odd mb (only
                                # after mb 2's deferred norm, which writes the
                                # otn regions outproj reads)
                                if mb % 2 == 1 and mb > 2 and opq:
                                    opq.pop(0)()
                                if mb == 14 and len(opq) > 6:
                                    opq.pop(0)()
                            attn_mm(*pend)
                            # normalization: spill O rows; 1/den via ln per
                            # head + one combined exp(-x) on ACT writing bf16
                            # (feeds the K=1 broadcast matmul); frees both
                            # "ot" banks right away, broadcast + one DVE mul
                            # deferred into the next quarter.
                            osb = at.tile([64, 2, 512], F32, tag="ots", bufs=4)
                            recbs = []
                            for j in range(2):
                                lnd = at.tile(
                                    [1, 512], F32, tag="lnd", bufs=4,
                                    name=f"lnd_{j}",
                                )
                                nc.scalar.activation(
                                    lnd[:], ot_ab[j][64:65, :], LOG
                                )
                                nc.vector.tensor_copy(
                                    osb[:, j, :], ot_ab[j][0:64, :]
                                )
                                recb = at.tile(
                                    [1, 512], BF16, tag="recb", bufs=4,
                                    name=f"recb_{j}",
                                )
                                nc.scalar.activation(
                                    recb[:], lnd[:], EXP, scale=-1.0
                                )
                                recbs.append(recb)

                            def make_norm(osb_l, recb_l, p_l, qi_l):
                                def go():
                                    for j in range(2):
                                        bct = ps.tile(
                                            [128, 512], F32, tag="aux",
                                            bufs=2, name=f"bct{j}",
                                        )
                                        nc.tensor.matmul(
                                            bct[0:64, :],
                                            ones1[0:1, :],
                                            recb_l[j][:],
                                            start=True,
                                            stop=True,
                                        )
                                        nc.vector.tensor_mul(
                                            otn[p_l][
                                                64 * j : 64 * (j + 1), qi_l, :
                                            ],
                                            osb_l[:, j, :],
                                            bct[0:64, :],
                                        )
                                return go

                            pending_norm["fn"] = make_norm(
                                osb, recbs, p, f * 2 + sub
                            )
                            if p == PAIRS - 1:
                                last = f == 1 and sub == 1
                                if last:
                                    pending_norm["fn"]()
                                    pending_norm["fn"] = None
                                outproj_quarter(f * 2 + sub, final=last)
                                if last:
                                    while opq:
                                        opq.pop(0)()

    return nc


_CACHED = {}


def _get_kernel():
    if "nc" not in _CACHED:
        _CACHED["nc"] = build_kernel()
    return _CACHED["nc"]


def kernel(x, rotary_emb_x, Wq, Wkv, Wo, bo):
    import ml_dtypes
    from concourse.bass_utils import run_bass_kernel_spmd

    x = np.asarray(x, np.float32)
    rope = np.asarray(rotary_emb_x, np.float32)
    Wq = np.asarray(Wq, np.float32).astype(ml_dtypes.bfloat16)
    Wkv = np.asarray(Wkv, np.float32).astype(ml_dtypes.bfloat16)
    Wo = np.asarray(Wo, np.float32).astype(ml_dtypes.bfloat16)
    bo = np.asarray(bo, np.float32)

    cosT = np.ascontiguousarray(np.cos(rope).T)  # [64, N]
    sinT = np.ascontiguousarray(np.sin(rope).T)
    cosT2 = np.ascontiguousarray(np.concatenate([cosT, cosT], axis=0))
    sinT2 = np.concatenate([sinT, sinT], axis=0)
    # fold rotate_half's sign into sin: the low half of each 64-row head
    # block multiplies -q_hi
    sinT2 = sinT2.copy()
    sinT2[0:32] = -sinT2[0:32]
    sinT2[64:96] = -sinT2[64:96]
    sinT2 = np.ascontiguousarray(sinT2)

    Wk_full = Wkv[:, : H * DH]
    Wv_full = Wkv[:, H * DH :]

    xTs = [np.ascontiguousarray(x[b].T) for b in range(B)]
    in_maps = []
    for core in range(N_CORES):
        b, hg = divmod(core, 2)
        isl = slice(hg * INNER, (hg + 1) * INNER)
        in_maps.append(
            {
                "xT": xTs[b],
                "wq": np.ascontiguousarray(Wq[:, isl]),
                "wk": np.ascontiguousarray(Wk_full[:, isl]),
                "wv": np.ascontiguousarray(Wv_full[:, isl]),
                "wo": np.ascontiguousarray(Wo[isl, :]),
                "cosT": cosT2,
                "sinT": sinT2,
            }
        )

    nc = _get_kernel()
    _CACHED["in_maps"] = in_maps
    res = run_bass_kernel_spmd(nc, in_maps, list(range(N_CORES)))
    outs = [res.results[i]["out"] for i in range(N_CORES)]
    full = np.stack(
        [outs[2 * b] + outs[2 * b + 1] + bo for b in range(B)], axis=0
    )
    return full
